# revision 25
# baseline (speedup 1.0000x reference)
"""Trainium2 kernel for nn_LinearDynamics: chunked two-level scan, 8-core data parallel.

v8: fp8 DoubleRow u-path (0.5 cyc/row) + PSUM-resident state + bf16 output.

Per core (128 batch rows, state transposed xT [d_x=128, b=128]):
  x_{t+1} = x_t + (x_t @ dtA + u_t @ B2),  dtA = dt*A, B2 = dt*B, M = I + dtA

Chunks: S=16 chunks of L=16 steps, grouped NG=4 x GS=4 (GW=512 cols).

u layout: 3D [64(du), 2(parity), g*4096 + i*512 + q*128 + b] fp8, where
timestep t = (4g+q)*16 + 2i+par. Shared by phases A and C via DoubleRow
matmuls (contraction over (du, parity) pairs):
  A stationary slot i: sub0 = 128*N_{15-2i}, sub1 = 128*N_{14-2i} (fp8)
  C stationary par p:  sub_p = 128*B2, other sub = 0 (fp8) -> injects
    exactly one timestep per step despite the pair-packed moving data.
All weights carry a x128 scale (fp8 range); dtA/MP are 128*dtA etc in
bf16 (exact), so PSUM holds 128*x and every psum->bf16 output copy
applies scale 1/128 (DVE tensor_scalar_mul / ACT activation scale).

Phase A: W_s into PSUM (8 DoubleRow matmuls/group, ~107ns each);
  ACT copy psum*(1/128) -> Wext bf16.
Phase B': boundary X-block(g) into psC[g]: 4 windowed W-terms (+ chain
  X-block(g-1) @ 128*M^{4L} via the bf16 xr0 copy). Group left OPEN.
Phase C per step: 4 DoubleRow u-matmuls (shared stationary), then 4
  bf16 x-matmuls (128*dtA), then per-group scaled copy psum -> bf16 xr
  (= next matmul input AND DMA output). 2-step output slabs; last
  slab's descs split sync/scalar after the final copies.
Host-sim rel err ~4.4e-3.
"""

import ml_dtypes
import numpy as np

DT = 0.1
BATCH, T, DX, DU = 1024, 256, 128, 64
NCORES = 8
BPC = BATCH // NCORES  # 128
S, L = 16, 16
NG, GS = 4, 4
GW = GS * BPC  # 512
SC = 128.0  # fp8 weight scale, folded out in the output copies

_CACHE = {}


def _build(debug=False):
    import concourse.mybir as mybir
    import concourse.tile as tile
    from concourse import bacc

    f32 = mybir.dt.float32
    bf16 = mybir.dt.bfloat16
    fp8 = mybir.dt.float8e4
    DR = mybir.MatmulPerfMode.DoubleRow

    nc = bacc.Bacc("TRN2", target_bir_lowering=False, debug=debug)
    w8_d = nc.declare_dram_parameter("W8", [DU, 2, 10 * DX], fp8, isOutput=False)
    wa_d = nc.declare_dram_parameter("WA", [DX, DX], bf16, isOutput=False)
    mp_d = nc.declare_dram_parameter("MP", [DX, 5 * DX], bf16, isOutput=False)
    w0_d = nc.declare_dram_parameter("W0T", [DX, 4 * DX], bf16, isOutput=False)
    u_d = nc.declare_dram_parameter("uT", [DU, 2, NG * 8 * GW], fp8, isOutput=False)
    y_d = nc.declare_dram_parameter("yT", [NG, 8, DX, 2 * GW], bf16, isOutput=True)

    with tile.TileContext(nc) as tc:
        with (
            tc.tile_pool(name="cw", bufs=1) as cw,
            tc.tile_pool(name="psA", bufs=2, space="PSUM") as psA,
            tc.tile_pool(name="psW", bufs=1, space="PSUM") as psW,
            tc.tile_pool(name="psC", bufs=1, space="PSUM") as psC,
        ):
            # PE clock warm-up while input DMAs are in flight
            scr = cw.tile([DX, GW], bf16)
            nc.gpsimd.memset(scr[:], 0)
            psw = psW.tile([DX, GW], f32)
            for _ in range(10):
                nc.tensor.matmul(psw[:], scr[:, 0:DX], scr[:], start=True, stop=True)

            # weights on the ACT queue, u on the SYNC queue
            W8 = cw.tile([DU, 2, 10 * DX], fp8)
            nc.scalar.dma_start(W8[:], w8_d[:])
            u3 = cw.tile([DU, 2, NG * 8 * GW], fp8)
            GB = 8 * GW  # 4096 pair-block cols per group in the 3D layout
            for h in range(2):
                nc.sync.dma_start(
                    u3[:, :, h * GB // 2 : (h + 1) * GB // 2],
                    u_d[:, :, h * GB // 2 : (h + 1) * GB // 2],
                )
            WA = cw.tile([DX, DX], bf16)
            nc.scalar.dma_start(WA[:], wa_d[:])
            MP = cw.tile([DX, 5 * DX], bf16)
            nc.scalar.dma_start(MP[:], mp_d[:])
            Wext = cw.tile([DX, (4 + S) * DX], bf16)
            nc.scalar.dma_start(Wext[:, 0 : 4 * DX], w0_d[:])
            for g in range(1, NG):
                nc.sync.dma_start(
                    u3[:, :, g * GB : (g + 1) * GB], u_d[:, :, g * GB : (g + 1) * GB]
                )

            xr = [
                cw.tile([DX, (L + 1) * GW], bf16, name=f"xr{g}") for g in range(NG)
            ]
            psCt = [psC.tile([DX, GW], f32, name=f"psCt{g}") for g in range(NG)]

            def ccopy(idx, dst, src):
                # psum*(1/SC) -> bf16, alternating DVE / ACT
                if idx % 2 == 0:
                    nc.vector.tensor_scalar_mul(dst, src, 1.0 / SC)
                else:
                    nc.scalar.mul(dst, src, 1.0 / SC)

            for g in range(NG):
                # phase A: W for the 4 chunks of group g (DoubleRow fp8)
                ps = psA.tile([DX, GW], f32)
                for i in range(8):
                    c0 = g * GB + i * GW
                    nc.tensor.matmul(
                        ps[:],
                        W8[:, :, i * DX : (i + 1) * DX],
                        u3[:, :, c0 : c0 + GW],
                        start=(i == 0),
                        stop=(i == 7),
                        perf_mode=DR,
                    )
                nc.scalar.mul(
                    Wext[:, (4 + g * GS) * DX : (4 + (g + 1) * GS) * DX],
                    ps[:],
                    1.0 / SC,
                )
                # phase B': windowed W part (+ chain term); leave group OPEN
                for d in range(4):
                    sc0 = (4 * g + 3 - d) * DX
                    nc.tensor.matmul(
                        psCt[g][:],
                        MP[:, d * DX : (d + 1) * DX],
                        Wext[:, sc0 : sc0 + GW],
                        start=(d == 0),
                        stop=False,
                    )
                if g > 0:
                    nc.tensor.matmul(
                        psCt[g][:],
                        MP[:, 4 * DX : 5 * DX],
                        xr[g - 1][:, 0:GW],
                        start=False,
                        stop=False,
                    )
                ccopy(g, xr[g][:, 0:GW], psCt[g][:])

            # phase C: psum IS the (x128-scaled) state.
            # Per step: 4 DoubleRow u-matmuls (one shared stationary), then
            # 4 bf16 x-matmuls, then the per-group scaled output copies.
            ci = 0
            for k in range(1, L + 1):
                j = k - 1
                par = j & 1
                i = j >> 1
                for g in range(NG):
                    c0 = g * GB + i * GW
                    nc.tensor.matmul(
                        psCt[g][:],
                        W8[:, :, (8 + par) * DX : (9 + par) * DX],
                        u3[:, :, c0 : c0 + GW],
                        start=False,
                        stop=False,
                        perf_mode=DR,
                    )
                for g in range(NG):
                    nc.tensor.matmul(
                        psCt[g][:],
                        WA[:],
                        xr[g][:, (k - 1) * GW : k * GW],
                        start=False,
                        stop=(k == L),
                    )
                    ccopy(ci, xr[g][:, k * GW : (k + 1) * GW], psCt[g][:])
                    ci += 1
                if k % 2 == 0:
                    m = k // 2 - 1
                    for g in range(NG):
                        eng = nc.sync if (k < L or g < 2) else nc.scalar
                        eng.dma_start(
                            y_d[g][m],
                            xr[g][:, (2 * m + 1) * GW : (2 * m + 3) * GW],
                        )
    nc.compile()
    return nc


def _get_nc():
    if "nc" not in _CACHE:
        _CACHE["nc"] = _build()
    return _CACHE["nc"]


def _host_mats(A, Bmat):
    M64 = np.eye(DX, dtype=np.float64) + DT * A.astype(np.float64)
    B264 = DT * Bmat.astype(np.float64)
    Np = []
    Mp = np.eye(DX, dtype=np.float64)
    for p in range(L):
        Np.append((B264 @ Mp).astype(np.float64))
        Mp = Mp @ M64
    ML64 = Mp  # M^L
    # fp8 stationary pack [du, 2, 10*DX], all x128
    W8 = np.zeros((DU, 2, 10 * DX), dtype=np.float64)
    for i in range(8):
        W8[:, 0, i * DX : (i + 1) * DX] = Np[15 - 2 * i]
        W8[:, 1, i * DX : (i + 1) * DX] = Np[14 - 2 * i]
    W8[:, 0, 8 * DX : 9 * DX] = B264
    W8[:, 1, 9 * DX : 10 * DX] = B264
    W8 = (SC * W8).astype(np.float32).astype(ml_dtypes.float8_e4m3)
    WA = (SC * DT * A.astype(np.float64)).astype(np.float32)
    MP = np.zeros((DX, 5 * DX), dtype=np.float32)
    Md = np.eye(DX, dtype=np.float64)
    for d in range(5):
        MP[:, d * DX : (d + 1) * DX] = (SC * Md).astype(np.float32)
        Md = Md @ ML64
    return (
        W8,
        WA.astype(ml_dtypes.bfloat16),
        MP.astype(ml_dtypes.bfloat16),
    )


def _prep_inputs(initial_state, u_traj, A, Bmat):
    W8, WA, MP = _host_mats(A, Bmat)
    in_maps = []
    for c in range(NCORES):
        rc = slice(c * BPC, (c + 1) * BPC)
        w0 = np.zeros((DX, 4 * DX), dtype=np.float32)
        w0[:, 3 * DX :] = initial_state[rc].T
        uc = u_traj[rc]  # [b, t, du]; t = (4g+q)*16 + 2i+par
        ut = uc.reshape(BPC, NG, GS, 8, 2, DU)  # b, g, q, i, par, du
        ut = ut.transpose(5, 4, 1, 3, 2, 0)  # du, par, g, i, q, b
        uT = (
            np.ascontiguousarray(ut)
            .reshape(DU, 2, NG * 8 * GW)
            .astype(ml_dtypes.float8_e4m3)
        )
        in_maps.append(
            {
                "W8": W8,
                "WA": WA,
                "MP": MP,
                "W0T": w0.astype(ml_dtypes.bfloat16),
                "uT": uT,
            }
        )
    return in_maps


def _assemble(results, initial_state):
    out = np.empty((BATCH, T + 1, DX), dtype=np.float32)
    out[:, 0, :] = initial_state
    for c in range(NCORES):
        rc = slice(c * BPC, (c + 1) * BPC)
        yT = results[c]["yT"]  # [g, m, dx, kin*q*b] bf16
        y = np.asarray(yT).reshape(NG, 8, DX, 2, GS, BPC)  # g, m, dx, kin, q, b
        y = y.transpose(5, 0, 4, 1, 3, 2)  # b, g, q, m, kin, dx
        out[rc, 1:, :] = y.reshape(BPC, T, DX).astype(np.float32)
    return out


def run(initial_state, u_traj, A, Bmat, trace=False, **trace_kwargs):
    from concourse.bass_utils import run_bass_kernel_spmd

    nc = _get_nc()
    in_maps = _prep_inputs(initial_state, u_traj, A, Bmat)
    res = run_bass_kernel_spmd(
        nc, in_maps, list(range(NCORES)), trace=trace, **trace_kwargs
    )
    out = _assemble(res.results, initial_state)
    return out, res


def kernel(initial_state, u_traj, A, Bmat):
    out, _ = run(initial_state, u_traj, A, Bmat)
    return out


# revision 27
# speedup vs baseline: 1.4826x; 1.4826x over previous
"""Trainium2 kernel for nn_LinearDynamics: chunked two-level scan, 8-core data parallel.

v3: PSUM-resident state + all-bf16 datapath + bf16 output
    + uniform 128-row stationaries (no PE tile-config switches)
    + group-chained boundary phase + parallel input DMA queues.

Per core (128 batch rows, state transposed xT [d_x=128, b=128]):
  x_{t+1} = x_t + (x_t @ dtA + u_t @ B2),  dtA = dt*A, B2 = dt*B, M = I + dtA

Chunks: S=16 chunks of L=16 steps, grouped NG=4 x GS=4 (GW=512 cols).
Host precomputes (float64, cast bf16):
  Wt slot i (i<8): rows 0:64 = N_{15-2i}, rows 64:128 = N_{14-2i}, N_p = B2@M^p
  Wt slot 8/9: B2 zero-padded to rows 0:64 / rows 64:128; slot 10: dtA
  MP_d = M^(d*L)  d=0..15

Phase A: W_s = sum_j u_{sL+j} @ N_{15-j}; u pair-packed on partitions,
  8 matmuls/group into PSUM; ACT copies psum -> Wext (bf16).
Phase B': boundary X-block(g) into psC[g]: 4 windowed W-terms (+ for g>0
  one chain matmul X-block(g-1) @ M^{4L} using the bf16 xr0 copy).
  Group accumulation left OPEN.
Phase C: PE keeps accumulating into the same psum bank:
    psC[g] += u_j @ B2pad + xr_{k-1} @ dtA     (psum IS the f32 state)
  one copy per step (DVE/ACT alternate) -> bf16 xr tile = next matmul
  input AND DMA output ([128,1024] slabs every 2 steps, host->f32).

All stationaries are full 128-row (128,128) tiles so LDWEIGHTS pipelines
with the previous matmul (~215ns/matmul at 2.4GHz).
Host-sim rel err ~3.8e-3.
"""

import ml_dtypes
import numpy as np

DT = 0.1
BATCH, T, DX, DU = 1024, 256, 128, 64
NCORES = 8
BPC = BATCH // NCORES  # 128
S, L = 16, 16
NG, GS = 4, 4
GW = GS * BPC  # 512

_CACHE = {}


def _build(debug=False):
    import concourse.mybir as mybir
    import concourse.tile as tile
    from concourse import bacc

    f32 = mybir.dt.float32
    bf16 = mybir.dt.bfloat16
    fp8 = mybir.dt.float8e4

    nc = bacc.Bacc("TRN2", target_bir_lowering=False, debug=debug)
    wt_d = nc.declare_dram_parameter("WT", [DX, 11 * DX], bf16, isOutput=False)
    mp_d = nc.declare_dram_parameter("MP", [DX, 5 * DX], bf16, isOutput=False)
    w0_d = nc.declare_dram_parameter("W0T", [DX, 4 * DX], bf16, isOutput=False)
    u_d = nc.declare_dram_parameter("uT", [NG, DX, 8 * GW], fp8, isOutput=False)
    y_d = nc.declare_dram_parameter("yT", [NG, 8, DX, 2 * GW], bf16, isOutput=True)

    with tile.TileContext(nc) as tc:
        with (
            tc.tile_pool(name="cw", bufs=1) as cw,
            tc.tile_pool(name="psA", bufs=2, space="PSUM") as psA,
            tc.tile_pool(name="psW", bufs=1, space="PSUM") as psW,
            tc.tile_pool(name="psC", bufs=1, space="PSUM") as psC,
        ):
            # PE clock warm-up: dummy matmuls on zeroed scratch run while the
            # input DMAs are still in flight, so the PE reaches max p-state
            # (needs ~3us continuous busy) before real work starts.
            scr = cw.tile([DX, GW], bf16)
            nc.gpsimd.memset(scr[:], 0)
            psw = psW.tile([DX, GW], f32)
            for _ in range(10):
                nc.tensor.matmul(psw[:], scr[:, 0:DX], scr[:], start=True, stop=True)

            # weights go on the ACT queue, u on the SYNC queue (parallel
            # descriptor processing; transfers share HBM). Wt is split so
            # the first matmul gates on only slots 0-3 + the first u quarter.
            Wt = cw.tile([DX, 11 * DX], bf16)
            nc.scalar.dma_start(Wt[:, 0 : 4 * DX], wt_d[:, 0 : 4 * DX])
            nc.scalar.dma_start(Wt[:, 4 * DX :], wt_d[:, 4 * DX :])
            u_sb = cw.tile([DX, NG * 8 * GW], fp8)
            # u group 0 in halves so phase A starts ASAP; later groups in
            # single DMAs (desc processing is ~0.65us per DMA on the queue)
            for h in range(2):
                nc.sync.dma_start(
                    u_sb[:, h * 4 * GW : (h + 1) * 4 * GW],
                    u_d[0][:, h * 4 * GW : (h + 1) * 4 * GW],
                )
            MP = cw.tile([DX, 5 * DX], bf16)
            nc.scalar.dma_start(MP[:], mp_d[:])
            Wext = cw.tile([DX, (4 + S) * DX], bf16)
            nc.scalar.dma_start(Wext[:, 0 : 4 * DX], w0_d[:])
            for g in range(1, NG):
                c0 = g * 8 * GW
                nc.sync.dma_start(u_sb[:, c0 : c0 + 8 * GW], u_d[g][:])

            xr = [
                cw.tile([DX, (L + 1) * GW], bf16, name=f"xr{g}") for g in range(NG)
            ]
            psCt = [psC.tile([DX, GW], f32, name=f"psCt{g}") for g in range(NG)]

            def ccopy(idx, dst, src):
                # alternate DVE / ACT for the per-step psum->bf16 copy
                if idx % 2 == 0:
                    nc.vector.tensor_copy(dst, src)
                else:
                    nc.scalar.copy(dst, src)

            for g in range(NG):
                # phase A: W for the 4 chunks of group g
                ps = psA.tile([DX, GW], f32)
                for i in range(8):
                    nc.tensor.matmul(
                        ps[:],
                        Wt[:, i * DX : (i + 1) * DX],
                        u_sb[:, g * 8 * GW + i * GW : g * 8 * GW + (i + 1) * GW],
                        start=(i == 0),
                        stop=(i == 7),
                    )
                nc.scalar.copy(
                    Wext[:, (4 + g * GS) * DX : (4 + (g + 1) * GS) * DX], ps[:]
                )
                # phase B': windowed W part (+ chain term for g>0); leave OPEN
                for d in range(4):
                    sc = (4 * g + 3 - d) * DX
                    nc.tensor.matmul(
                        psCt[g][:],
                        MP[:, d * DX : (d + 1) * DX],
                        Wext[:, sc : sc + GW],
                        start=(d == 0),
                        stop=False,
                    )
                if g > 0:
                    nc.tensor.matmul(
                        psCt[g][:],
                        MP[:, 4 * DX : 5 * DX],
                        xr[g - 1][:, 0:GW],
                        start=False,
                        stop=False,
                    )
                ccopy(g, xr[g][:, 0:GW], psCt[g][:])

            # phase C: psum IS the state; one copy per step per group
            ci = 0
            for k in range(1, L + 1):
                j = k - 1
                par = j & 1
                i = j >> 1
                for g in range(NG):
                    nc.tensor.matmul(
                        psCt[g][:],
                        Wt[:, (8 + par) * DX : (9 + par) * DX],
                        u_sb[:, g * 8 * GW + i * GW : g * 8 * GW + (i + 1) * GW],
                        start=False,
                        stop=False,
                    )
                    nc.tensor.matmul(
                        psCt[g][:],
                        Wt[:, 10 * DX : 11 * DX],
                        xr[g][:, (k - 1) * GW : k * GW],
                        start=False,
                        stop=(k == L),
                    )
                    ccopy(ci, xr[g][:, k * GW : (k + 1) * GW], psCt[g][:])
                    ci += 1
                    # 2-step output slabs; the last slab goes out per-step
                    # (smaller final transfers, descs spread across queues)
                    if k % 2 == 0 and k <= 14:
                        m = k // 2 - 1
                        nc.sync.dma_start(
                            y_d[g][m],
                            xr[g][:, (2 * m + 1) * GW : (2 * m + 3) * GW],
                        )
                    elif k == 15:
                        nc.sync.dma_start(
                            y_d[g][7][:, 0:GW], xr[g][:, 15 * GW : 16 * GW]
                        )
                # k=16 descs go AFTER all four final copies so a desc waiting
                # on one group's copy never blocks another group's copy on
                # the same engine queue
                if k == 16:
                    for g in range(NG):
                        eng = [nc.sync, nc.sync, nc.scalar, nc.scalar][g]
                        eng.dma_start(
                            y_d[g][7][:, GW : 2 * GW], xr[g][:, 16 * GW : 17 * GW]
                        )
    nc.compile()
    return nc


def _get_nc():
    if "nc" not in _CACHE:
        _CACHE["nc"] = _build()
    return _CACHE["nc"]


def _host_mats(A, Bmat):
    M64 = np.eye(DX, dtype=np.float64) + DT * A.astype(np.float64)
    B264 = DT * Bmat.astype(np.float64)
    Np = []
    Mp = np.eye(DX, dtype=np.float64)
    for p in range(L):
        Np.append((B264 @ Mp).astype(np.float32))
        Mp = Mp @ M64
    ML64 = Mp  # M^L
    Wt = np.zeros((DX, 11 * DX), dtype=np.float32)
    for i in range(8):
        Wt[0:DU, i * DX : (i + 1) * DX] = Np[15 - 2 * i]
        Wt[DU : 2 * DU, i * DX : (i + 1) * DX] = Np[14 - 2 * i]
    B2 = B264.astype(np.float32)
    Wt[0:DU, 8 * DX : 9 * DX] = B2
    Wt[DU : 2 * DU, 9 * DX : 10 * DX] = B2
    Wt[:, 10 * DX : 11 * DX] = (DT * A.astype(np.float64)).astype(np.float32)
    MP = np.zeros((DX, 5 * DX), dtype=np.float32)
    Md = np.eye(DX, dtype=np.float64)
    for d in range(5):
        MP[:, d * DX : (d + 1) * DX] = Md.astype(np.float32)
        Md = Md @ ML64
    return Wt.astype(ml_dtypes.bfloat16), MP.astype(ml_dtypes.bfloat16)


def _prep_inputs(initial_state, u_traj, A, Bmat):
    Wt, MP = _host_mats(A, Bmat)
    in_maps = []
    for c in range(NCORES):
        rc = slice(c * BPC, (c + 1) * BPC)
        w0 = np.zeros((DX, 4 * DX), dtype=np.float32)
        w0[:, 3 * DX :] = initial_state[rc].T
        uc = u_traj[rc]  # [b, t, du]; t = (4g+q)*16 + 2i+par
        ut = uc.reshape(BPC, NG, GS, 8, 2, DU)  # b, g, q, i, par, du
        ut = ut.transpose(1, 4, 5, 3, 2, 0)  # g, par, du, i, q, b
        uT = (
            np.ascontiguousarray(ut)
            .reshape(NG, DX, 8 * GW)
            .astype(ml_dtypes.float8_e4m3)
        )
        in_maps.append(
            {
                "WT": Wt,
                "MP": MP,
                "W0T": w0.astype(ml_dtypes.bfloat16),
                "uT": uT,
            }
        )
    return in_maps


def _assemble(results, initial_state):
    out = np.empty((BATCH, T + 1, DX), dtype=np.float32)
    out[:, 0, :] = initial_state
    for c in range(NCORES):
        rc = slice(c * BPC, (c + 1) * BPC)
        yT = results[c]["yT"]  # [g, m, dx, kin*q*b] bf16
        y = np.asarray(yT).reshape(NG, 8, DX, 2, GS, BPC)  # g, m, dx, kin, q, b
        y = y.transpose(5, 0, 4, 1, 3, 2)  # b, g, q, m, kin, dx
        out[rc, 1:, :] = y.reshape(BPC, T, DX).astype(np.float32)
    return out


def run(initial_state, u_traj, A, Bmat, trace=False, **trace_kwargs):
    from concourse.bass_utils import run_bass_kernel_spmd

    nc = _get_nc()
    in_maps = _prep_inputs(initial_state, u_traj, A, Bmat)
    res = run_bass_kernel_spmd(
        nc, in_maps, list(range(NCORES)), trace=trace, **trace_kwargs
    )
    out = _assemble(res.results, initial_state)
    return out, res


def kernel(initial_state, u_traj, A, Bmat):
    out, _ = run(initial_state, u_traj, A, Bmat)
    return out
